# revision 21
# baseline (speedup 1.0000x reference)
"""Lennard-Jones pair energies + per-atom segment sum on 8 Trainium2 cores.

Strategy (edge-partitioned per the sharding hint, ELL-style dense layout):

Host (sharding step): atoms are sorted by padded pair count and grouped into
chunks of 1024 (8 cores x 128 partitions); chunk i keeps L_i = max padded
count in the chunk (plus 2 fixup slots, L even), so there are no pad atoms
and slot padding is minimal.  Each core receives a partition-major fp16
buffer [128, F_total]; every DMA is contiguous per partition.  Pad slots use
dist=RC (shifted LJ energy exactly 0).  The two fixup slots per chunk carry
host-computed distances whose pair energies sum to the column's additive
constant -L*e0/2, so the device-side reduce alone yields the final per-atom
energy.  Within each device tile, columns are packed *folded*: the first
half of every chunk's slots in the tile's left half, the second half
mirrored in the right half, so one full-width tensor_tensor add folds the
tile 2:1 before the (1x-rate) grouped reduce.

Device: one activation-table preload (ln/exp share a table set), then per
tile: contiguous DMA, ACT ln (fp16->f32), ACT exp -> v = sqrt2*d^-6 (fp16),
DVE tensor_scalar u = v - 2b (4x), tensor_tensor bp = u*v (2x, partly on
GpSimd), tensor_tensor fold add (2x, partly on GpSimd), DVE grouped
tensor_reduce per equal-L chunk run, per-tile output DMA of [128, m] f32.

Host (unshard step): scatters per-atom results back to atom order.
"""

import math

import numpy as np

RC = 3.0
N_CORES = 8
P = 128
CH = N_CORES * P  # atoms per chunk
PAD_MULT = 2  # per-atom slot-count quantum
N_FIX = 2  # fixup slots per chunk (keeps padded width even for the fold)

_E0 = 4.0 * ((1.0 / RC) ** 12 - (1.0 / RC) ** 6)
_B = math.sqrt(0.5)

# cumulative tile boundaries as fractions of total width (small first tile
# for pipeline ramp, small last tile for a short tail)
TILE_FRACS = [0.04, 0.14, 0.32, 0.54, 0.78, 1.0]
GP_TT = 0.30  # fraction of the bp multiply given to GpSimd
GP_FOLD = 0.35  # fraction of the fold add given to GpSimd


def _merge_runs(Lc: np.ndarray, max_runs: int = 7, max_cost: int = 60000):
    """Round some chunks' L up to the next-larger run's L to cut the number
    of distinct L values. Lc is non-increasing (sorted desc)."""
    Lc = Lc.copy()
    while True:
        uniq = sorted(set(int(x) for x in Lc), reverse=True)
        if len(uniq) <= max_runs:
            break
        best = None
        for i in range(1, len(uniq)):
            src = uniq[i]
            dst = uniq[i - 1]
            m = int(np.sum(Lc == src))
            cost = m * CH * (dst - src)
            if best is None or cost < best[0]:
                best = (cost, src, dst)
        if best[0] > max_cost:
            break
        Lc[Lc == best[1]] = best[2]
    return Lc


def _chunk_geometry(idx: np.ndarray, n_atoms: int):
    counts = np.bincount(idx, minlength=n_atoms).astype(np.int64)
    perm = np.argsort(idx, kind="stable")
    starts = np.zeros(n_atoms + 1, np.int64)
    starts[1:] = np.cumsum(counts)
    q = ((counts + PAD_MULT - 1) // PAD_MULT) * PAD_MULT
    order = np.argsort(-q, kind="stable")
    n_chunks = (n_atoms + CH - 1) // CH
    n_pad = n_chunks * CH
    order_pad = np.full(n_pad, -1, np.int64)
    order_pad[:n_atoms] = order
    qs = np.where(order_pad >= 0, q[np.maximum(order_pad, 0)], 0)
    Lc = np.maximum(qs.reshape(n_chunks, CH).max(axis=1), PAD_MULT)
    Lc = _merge_runs(Lc)
    Lp = Lc + N_FIX  # even
    return counts, perm, starts, order_pad, Lc, Lp, n_chunks


def _tile_plan(Lp):
    """Group chunks into device tiles at TILE_FRACS boundaries.

    Returns list of tiles (col_start, F, runs);
    runs = [(w_off, Lh, m, out_col)] over the tile's FOLDED layout, where
    Lh = Lp/2 and w_off is the column offset inside the folded half-width.
    """
    n = len(Lp)
    total = sum(Lp)
    bounds = []
    c0 = 0
    col = 0
    fi = 0
    for i in range(n):
        col += Lp[i]
        if fi < len(TILE_FRACS) - 1 and col >= TILE_FRACS[fi] * total:
            bounds.append((c0, i + 1))
            c0 = i + 1
            fi += 1
    if c0 < n:
        bounds.append((c0, n))
    tiles = []
    col = 0
    for c0, c1 in bounds:
        runs = []
        off = 0  # offset in folded half-width units
        j = c0
        while j < c1:
            k = j
            while k < c1 and Lp[k] == Lp[j]:
                k += 1
            runs.append((off, Lp[j] // 2, k - j, j))
            off += (Lp[j] // 2) * (k - j)
            j = k
        tiles.append((col, 2 * off, runs))
        col += 2 * off
    return tiles


def _build_layout(idx: np.ndarray, n_atoms: int, dist: np.ndarray):
    """Pack pairs into per-core partition-major fp16 tiles (folded order).

    Returns (packed, atom_of, Lp, n_chunks, tiles).
    """
    counts, perm, starts, order_pad, Lc, Lp, n_chunks = _chunk_geometry(
        idx, n_atoms
    )
    tiles = _tile_plan([int(x) for x in Lp])
    F_total = sum(F for _, F, _ in tiles)

    # fixup distances: each of the two slots contributes bp = -Lc*e0/4
    vfix = _B + np.sqrt(0.5 - Lc * _E0 / 4.0)
    dfix = (math.sqrt(2.0) / vfix) ** (1.0 / 6.0)

    dist_sorted = dist[perm].astype(np.float16)
    packed = np.full((N_CORES, P, F_total), np.float16(RC), np.float16)
    Lmax = int(Lc.max())
    offs_max = np.arange(Lmax)
    for tcol, Ft, runs in tiles:
        half = Ft // 2
        for w_off, Lh, m, j0 in runs:
            for j in range(j0, j0 + m):
                a = order_pad[j * CH : (j + 1) * CH]
                L = int(Lc[j])
                cnt = np.where(a >= 0, counts[np.maximum(a, 0)], 0)
                offs = offs_max[:L][None, :]
                valid = offs < cnt[:, None]
                src = starts[np.maximum(a, 0)][:, None] + offs
                block = np.full((CH, L + N_FIX), np.float16(RC), np.float16)
                block[:, :L][valid] = dist_sorted[src[valid]]
                block[:, L:] = np.float16(dfix[j])
                blk = block.reshape(N_CORES, P, L + N_FIX)
                o = tcol + w_off + (j - j0) * Lh
                packed[:, :, o : o + Lh] = blk[:, :, :Lh]
                packed[:, :, half + o : half + o + Lh] = blk[:, :, Lh:]
    atom_of = order_pad.reshape(n_chunks, N_CORES, P)
    return packed, atom_of, [int(x) for x in Lp], n_chunks, tiles


def _build_bass_program(tiles, F_total, n_chunks):
    import concourse.bass as bass
    import concourse.tile as tile
    from concourse import bacc, mybir

    f32 = mybir.dt.float32
    f16 = mybir.dt.float16
    AF = mybir.ActivationFunctionType
    OP = mybir.AluOpType

    nc = bacc.Bacc(
        "TRN2",
        target_bir_lowering=False,
        debug=False,
        enable_asserts=False,
        num_devices=N_CORES,
    )
    din = nc.dram_tensor("dist_packed", [P, F_total], f16, kind="ExternalInput")
    dout = nc.dram_tensor("en_out", [P, n_chunks], f32, kind="ExternalOutput")

    # activation table set holding ln+exp together (one load for the whole
    # program instead of a 1.3us reload per function switch)
    set_id = 6
    try:
        from concourse.hw_specs import get_activation_tables

        for i, (_, funcs) in enumerate(get_activation_tables("TRN2").items()):
            if AF.Ln in funcs and AF.Exp in funcs:
                set_id = i
                break
    except Exception:
        pass

    ln_sqrt2 = 0.5 * math.log(2.0)
    n_tiles = len(tiles)

    with tile.TileContext(nc) as tc:
        with (
            tc.tile_pool(name="io", bufs=3) as io_pool,
            tc.tile_pool(name="t", bufs=2) as tpool,
            tc.tile_pool(name="u", bufs=2) as upool,
            tc.tile_pool(name="w", bufs=2) as wpool,
            tc.tile_pool(name="acc", bufs=1) as acc_pool,
        ):
            atl = mybir.InstLoadActFuncSet(
                name=nc.get_next_instruction_name(),
                ins=[],
                outs=[],
                act_func_set_id=set_id,
            )
            nc.scalar.add_instruction(atl)
            out_raw = acc_pool.tile([P, n_chunks], f32, tag="out_raw")
            lbias = acc_pool.tile([P, 1], f32, tag="lbias")
            nc.vector.memset(lbias[:], ln_sqrt2)
            for ti, (col, F, runs) in enumerate(tiles):
                half = F // 2
                use_gp = ti < n_tiles - 1
                d = io_pool.tile([P, F], f16, tag="d")
                nc.sync.dma_start(d[:], din.ap()[:, col : col + F])
                # t = ln(d) at f32 (exp amplifies ln error 6x)
                t = tpool.tile([P, F], f32, tag="t")
                nc.scalar.activation(t[:], d[:], AF.Ln)
                # v = sqrt2*d^-6 in fp16, written back over d
                nc.scalar.activation(
                    d[:], t[:], AF.Exp, bias=lbias[:], scale=-6.0
                )
                v = d
                # bp = (v - 2b)*v ; en/2 = bp - e0/2 (constant folded into
                # the per-chunk fixup slots).  ts runs 4x, tt 2x; a slice
                # of tt goes to the otherwise-idle GpSimd engine.
                u = upool.tile([P, F], f16, tag="u")
                nc.vector.tensor_scalar(u[:], v[:], 2.0 * _B, None, OP.subtract)
                sp = (int(F * (1.0 - GP_TT)) & ~1) if use_gp else F
                nc.vector.tensor_tensor(
                    v[:, :sp], u[:, :sp], v[:, :sp], OP.mult
                )
                if sp < F:
                    nc.gpsimd.tensor_tensor(
                        v[:, sp:], u[:, sp:], v[:, sp:], OP.mult
                    )
                # fold 2:1 with a single full-width add (layout is mirrored)
                w = wpool.tile([P, half], f16, tag="w")
                hsp = (int(half * (1.0 - GP_FOLD)) & ~1) if use_gp else half
                nc.vector.tensor_tensor(
                    w[:, :hsp], v[:, :hsp], v[:, half : half + hsp], OP.add
                )
                if hsp < half:
                    nc.gpsimd.tensor_tensor(
                        w[:, hsp:], v[:, hsp:half], v[:, half + hsp :], OP.add
                    )
                c0 = runs[0][3]
                c1 = runs[-1][3] + runs[-1][2]
                for w_off, Lh, m, out_col in runs:
                    nc.vector.tensor_reduce(
                        out_raw[:, out_col : out_col + m],
                        w[:, w_off : w_off + m * Lh].rearrange(
                            "p (b l) -> p b l", l=Lh
                        ),
                        axis=mybir.AxisListType.X,
                        op=OP.add,
                    )
                nc.sync.dma_start(dout.ap()[:, c0:c1], out_raw[:, c0:c1])
    nc.compile()
    return nc


def _prepare(inputs):
    dist = np.ascontiguousarray(np.asarray(inputs["dist"], dtype=np.float32))
    ind_2 = np.asarray(inputs["ind_2"])
    n_atoms = int(np.asarray(inputs["ind_1"]).shape[0])
    idx = ind_2[:, 0].astype(np.int64)

    packed, atom_of, Lp, n_chunks, tiles = _build_layout(idx, n_atoms, dist)
    F_total = packed.shape[2]
    in_maps = [
        {"dist_packed": np.ascontiguousarray(packed[c])} for c in range(N_CORES)
    ]
    nc = _build_bass_program(tiles, F_total, n_chunks)
    return nc, in_maps, (atom_of, n_atoms)


def _finish(res, meta):
    atom_of, n_atoms = meta
    out_full = np.zeros(n_atoms, np.float32)
    for c in range(N_CORES):
        dev = res.results[c]["en_out"]  # [P, n_chunks]
        a = atom_of[:, c, :]  # [n_chunks, P]
        valid = a >= 0
        out_full[a[valid]] = dev.T[valid]
    return out_full


def kernel(**inputs) -> np.ndarray:
    nc, in_maps, meta = _prepare(inputs)

    from concourse import bass_utils

    res = bass_utils.run_bass_kernel_spmd(
        nc, in_maps, core_ids=list(range(N_CORES))
    )
    return _finish(res, meta)


# revision 22
# speedup vs baseline: 1.0041x; 1.0041x over previous
"""Lennard-Jones pair energies + per-atom segment sum on 8 Trainium2 cores.

Strategy (edge-partitioned per the sharding hint, ELL-style dense layout):

Host (sharding step): atoms are sorted by padded pair count and grouped into
chunks of 1024 (8 cores x 128 partitions); chunk i keeps L_i = max padded
count in the chunk (plus 2 fixup slots, L even), so there are no pad atoms
and slot padding is minimal.  Each core receives a partition-major fp16
buffer [128, F_total]; every DMA is contiguous per partition.  Pad slots use
dist=RC (shifted LJ energy exactly 0).  The two fixup slots per chunk carry
host-computed distances whose pair energies sum to the column's additive
constant -L*e0/2, so the device-side reduce alone yields the final per-atom
energy.  Within each device tile, columns are packed *folded*: the first
half of every chunk's slots in the tile's left half, the second half
mirrored in the right half, so one full-width tensor_tensor add folds the
tile 2:1 before the (1x-rate) grouped reduce.

Device: one activation-table preload (ln/exp share a table set), then per
tile: contiguous DMA, ACT ln (fp16->f32), ACT exp -> v = sqrt2*d^-6 (fp16),
DVE tensor_scalar u = v - 2b (4x), tensor_tensor bp = u*v (2x, partly on
GpSimd), tensor_tensor fold add (2x, partly on GpSimd), DVE grouped
tensor_reduce per equal-L chunk run, per-tile output DMA of [128, m] f32.

Host (unshard step): scatters per-atom results back to atom order.
"""

import math

import numpy as np

RC = 3.0
N_CORES = 8
P = 128
CH = N_CORES * P  # atoms per chunk
PAD_MULT = 2  # per-atom slot-count quantum
N_FIX = 2  # fixup slots per chunk (keeps padded width even for the fold)

_E0 = 4.0 * ((1.0 / RC) ** 12 - (1.0 / RC) ** 6)
_B = math.sqrt(0.5)

# cumulative tile boundaries as fractions of total width (small first tile
# for pipeline ramp, small last tile for a short tail)
TILE_FRACS = [0.04, 0.14, 0.32, 0.54, 0.78, 1.0]
GP_TT = 0.30  # fraction of the bp multiply given to GpSimd
GP_FOLD = 0.35  # fraction of the fold add given to GpSimd


def _merge_runs(Lc: np.ndarray, max_runs: int = 7, max_cost: int = 60000):
    """Round some chunks' L up to the next-larger run's L to cut the number
    of distinct L values. Lc is non-increasing (sorted desc)."""
    Lc = Lc.copy()
    while True:
        uniq = sorted(set(int(x) for x in Lc), reverse=True)
        if len(uniq) <= max_runs:
            break
        best = None
        for i in range(1, len(uniq)):
            src = uniq[i]
            dst = uniq[i - 1]
            m = int(np.sum(Lc == src))
            cost = m * CH * (dst - src)
            if best is None or cost < best[0]:
                best = (cost, src, dst)
        if best[0] > max_cost:
            break
        Lc[Lc == best[1]] = best[2]
    return Lc


def _chunk_geometry(idx: np.ndarray, n_atoms: int):
    counts = np.bincount(idx, minlength=n_atoms).astype(np.int64)
    perm = np.argsort(idx, kind="stable")
    starts = np.zeros(n_atoms + 1, np.int64)
    starts[1:] = np.cumsum(counts)
    q = ((counts + PAD_MULT - 1) // PAD_MULT) * PAD_MULT
    order = np.argsort(-q, kind="stable")
    n_chunks = (n_atoms + CH - 1) // CH
    n_pad = n_chunks * CH
    order_pad = np.full(n_pad, -1, np.int64)
    order_pad[:n_atoms] = order
    qs = np.where(order_pad >= 0, q[np.maximum(order_pad, 0)], 0)
    Lc = np.maximum(qs.reshape(n_chunks, CH).max(axis=1), PAD_MULT)
    Lc = _merge_runs(Lc)
    Lp = Lc + N_FIX  # even
    return counts, perm, starts, order_pad, Lc, Lp, n_chunks


def _tile_plan(Lp):
    """Group chunks into device tiles at TILE_FRACS boundaries.

    Returns list of tiles (col_start, F, runs);
    runs = [(w_off, Lh, m, out_col)] over the tile's FOLDED layout, where
    Lh = Lp/2 and w_off is the column offset inside the folded half-width.
    """
    n = len(Lp)
    total = sum(Lp)
    bounds = []
    c0 = 0
    col = 0
    fi = 0
    for i in range(n):
        col += Lp[i]
        if fi < len(TILE_FRACS) - 1 and col >= TILE_FRACS[fi] * total:
            bounds.append((c0, i + 1))
            c0 = i + 1
            fi += 1
    if c0 < n:
        bounds.append((c0, n))
    tiles = []
    col = 0
    for c0, c1 in bounds:
        runs = []
        off = 0  # offset in folded half-width units
        j = c0
        while j < c1:
            k = j
            while k < c1 and Lp[k] == Lp[j]:
                k += 1
            runs.append((off, Lp[j] // 2, k - j, j))
            off += (Lp[j] // 2) * (k - j)
            j = k
        tiles.append((col, 2 * off, runs))
        col += 2 * off
    return tiles


def _build_layout(idx: np.ndarray, n_atoms: int, dist: np.ndarray):
    """Pack pairs into per-core partition-major fp16 tiles (folded order).

    Returns (packed, atom_of, Lp, n_chunks, tiles).
    """
    counts, perm, starts, order_pad, Lc, Lp, n_chunks = _chunk_geometry(
        idx, n_atoms
    )
    tiles = _tile_plan([int(x) for x in Lp])
    F_total = sum(F for _, F, _ in tiles)

    # fixup distances: each of the two slots contributes bp = -Lc*e0/4
    vfix = _B + np.sqrt(0.5 - Lc * _E0 / 4.0)
    dfix = (math.sqrt(2.0) / vfix) ** (1.0 / 6.0)

    dist_sorted = dist[perm].astype(np.float16)
    packed = np.full((N_CORES, P, F_total), np.float16(RC), np.float16)
    Lmax = int(Lc.max())
    offs_max = np.arange(Lmax)
    for tcol, Ft, runs in tiles:
        half = Ft // 2
        for w_off, Lh, m, j0 in runs:
            for j in range(j0, j0 + m):
                a = order_pad[j * CH : (j + 1) * CH]
                L = int(Lc[j])
                cnt = np.where(a >= 0, counts[np.maximum(a, 0)], 0)
                offs = offs_max[:L][None, :]
                valid = offs < cnt[:, None]
                src = starts[np.maximum(a, 0)][:, None] + offs
                block = np.full((CH, L + N_FIX), np.float16(RC), np.float16)
                block[:, :L][valid] = dist_sorted[src[valid]]
                block[:, L:] = np.float16(dfix[j])
                blk = block.reshape(N_CORES, P, L + N_FIX)
                o = tcol + w_off + (j - j0) * Lh
                packed[:, :, o : o + Lh] = blk[:, :, :Lh]
                packed[:, :, half + o : half + o + Lh] = blk[:, :, Lh:]
    atom_of = order_pad.reshape(n_chunks, N_CORES, P)
    return packed, atom_of, [int(x) for x in Lp], n_chunks, tiles


def _build_bass_program(tiles, F_total, n_chunks):
    import concourse.bass as bass
    import concourse.tile as tile
    from concourse import bacc, mybir

    f32 = mybir.dt.float32
    f16 = mybir.dt.float16
    AF = mybir.ActivationFunctionType
    OP = mybir.AluOpType

    nc = bacc.Bacc(
        "TRN2",
        target_bir_lowering=False,
        debug=False,
        enable_asserts=False,
        num_devices=N_CORES,
    )
    din = nc.dram_tensor("dist_packed", [P, F_total], f16, kind="ExternalInput")
    dout = nc.dram_tensor("en_out", [P, n_chunks], f32, kind="ExternalOutput")

    # activation table set holding ln+exp together (one load for the whole
    # program instead of a 1.3us reload per function switch)
    set_id = 6
    try:
        from concourse.hw_specs import get_activation_tables

        for i, (_, funcs) in enumerate(get_activation_tables("TRN2").items()):
            if AF.Ln in funcs and AF.Exp in funcs:
                set_id = i
                break
    except Exception:
        pass

    ln_sqrt2 = 0.5 * math.log(2.0)
    n_tiles = len(tiles)

    with tile.TileContext(nc) as tc:
        with (
            tc.tile_pool(name="io", bufs=5) as io_pool,
            tc.tile_pool(name="t", bufs=3) as tpool,
            tc.tile_pool(name="u", bufs=3) as upool,
            tc.tile_pool(name="w", bufs=3) as wpool,
            tc.tile_pool(name="acc", bufs=1) as acc_pool,
        ):
            atl = mybir.InstLoadActFuncSet(
                name=nc.get_next_instruction_name(),
                ins=[],
                outs=[],
                act_func_set_id=set_id,
            )
            nc.scalar.add_instruction(atl)
            out_raw = acc_pool.tile([P, n_chunks], f32, tag="out_raw")
            lbias = acc_pool.tile([P, 1], f32, tag="lbias")
            nc.vector.memset(lbias[:], ln_sqrt2)
            for ti, (col, F, runs) in enumerate(tiles):
                half = F // 2
                use_gp = ti < n_tiles - 1
                d = io_pool.tile([P, F], f16, tag="d")
                nc.sync.dma_start(d[:], din.ap()[:, col : col + F])
                # t = ln(d) at f32 (exp amplifies ln error 6x)
                t = tpool.tile([P, F], f32, tag="t")
                nc.scalar.activation(t[:], d[:], AF.Ln)
                # v = sqrt2*d^-6 in fp16, written back over d
                nc.scalar.activation(
                    d[:], t[:], AF.Exp, bias=lbias[:], scale=-6.0
                )
                v = d
                # bp = (v - 2b)*v ; en/2 = bp - e0/2 (constant folded into
                # the per-chunk fixup slots).  ts runs 4x, tt 2x; a slice
                # of tt goes to the otherwise-idle GpSimd engine.
                u = upool.tile([P, F], f16, tag="u")
                nc.vector.tensor_scalar(u[:], v[:], 2.0 * _B, None, OP.subtract)
                sp = (int(F * (1.0 - GP_TT)) & ~1) if use_gp else F
                nc.vector.tensor_tensor(
                    v[:, :sp], u[:, :sp], v[:, :sp], OP.mult
                )
                if sp < F:
                    nc.gpsimd.tensor_tensor(
                        v[:, sp:], u[:, sp:], v[:, sp:], OP.mult
                    )
                # fold 2:1 with a single full-width add (layout is mirrored)
                w = wpool.tile([P, half], f16, tag="w")
                hsp = (int(half * (1.0 - GP_FOLD)) & ~1) if use_gp else half
                nc.vector.tensor_tensor(
                    w[:, :hsp], v[:, :hsp], v[:, half : half + hsp], OP.add
                )
                if hsp < half:
                    nc.gpsimd.tensor_tensor(
                        w[:, hsp:], v[:, hsp:half], v[:, half + hsp :], OP.add
                    )
                c0 = runs[0][3]
                c1 = runs[-1][3] + runs[-1][2]
                for w_off, Lh, m, out_col in runs:
                    nc.vector.tensor_reduce(
                        out_raw[:, out_col : out_col + m],
                        w[:, w_off : w_off + m * Lh].rearrange(
                            "p (b l) -> p b l", l=Lh
                        ),
                        axis=mybir.AxisListType.X,
                        op=OP.add,
                    )
                nc.sync.dma_start(dout.ap()[:, c0:c1], out_raw[:, c0:c1])
    nc.compile()
    return nc


def _prepare(inputs):
    dist = np.ascontiguousarray(np.asarray(inputs["dist"], dtype=np.float32))
    ind_2 = np.asarray(inputs["ind_2"])
    n_atoms = int(np.asarray(inputs["ind_1"]).shape[0])
    idx = ind_2[:, 0].astype(np.int64)

    packed, atom_of, Lp, n_chunks, tiles = _build_layout(idx, n_atoms, dist)
    F_total = packed.shape[2]
    in_maps = [
        {"dist_packed": np.ascontiguousarray(packed[c])} for c in range(N_CORES)
    ]
    nc = _build_bass_program(tiles, F_total, n_chunks)
    return nc, in_maps, (atom_of, n_atoms)


def _finish(res, meta):
    atom_of, n_atoms = meta
    out_full = np.zeros(n_atoms, np.float32)
    for c in range(N_CORES):
        dev = res.results[c]["en_out"]  # [P, n_chunks]
        a = atom_of[:, c, :]  # [n_chunks, P]
        valid = a >= 0
        out_full[a[valid]] = dev.T[valid]
    return out_full


def kernel(**inputs) -> np.ndarray:
    nc, in_maps, meta = _prepare(inputs)

    from concourse import bass_utils

    res = bass_utils.run_bass_kernel_spmd(
        nc, in_maps, core_ids=list(range(N_CORES))
    )
    return _finish(res, meta)


# revision 25
# speedup vs baseline: 1.0950x; 1.0905x over previous
"""Lennard-Jones pair energies + per-atom segment sum on 8 Trainium2 cores.

Strategy (edge-partitioned per the sharding hint, ELL-style dense layout):

Host (sharding step): atoms are sorted by padded pair count and grouped into
chunks of 1024 (8 cores x 128 partitions); chunk i keeps L_i = max padded
count in the chunk (plus 2 fixup slots, L even), so there are no pad atoms
and slot padding is minimal.  Each core receives a partition-major fp16
buffer [128, F_total]; every DMA is contiguous per partition.  Pad slots use
dist=RC (shifted LJ energy exactly 0).  The two fixup slots per chunk carry
host-computed distances whose pair energies sum to the column's additive
constant -L*e0/2, so the device-side reduce alone yields the final per-atom
energy.  Within each device tile, columns are packed *folded*: the first
half of every chunk's slots in the tile's left half, the second half
mirrored in the right half, so one full-width tensor_tensor add folds the
tile 2:1 before the (1x-rate) grouped reduce.

Device: one activation-table preload (ln/exp share a table set), then per
tile: contiguous DMA, ACT ln (fp16->f32), ACT exp -> v = sqrt2*d^-6 (fp16),
DVE tensor_scalar u = v - 2b (4x), tensor_tensor bp = u*v (2x, partly on
GpSimd), tensor_tensor fold add (2x, partly on GpSimd), DVE grouped
tensor_reduce per equal-L chunk run, per-tile output DMA of [128, m] f32.

Host (unshard step): scatters per-atom results back to atom order.
"""

import math

import numpy as np

RC = 3.0
N_CORES = 8
P = 128
CH = N_CORES * P  # atoms per chunk
PAD_MULT = 2  # per-atom slot-count quantum
N_FIX = 2  # fixup slots per chunk (keeps padded width even for the fold)

_E0 = 4.0 * ((1.0 / RC) ** 12 - (1.0 / RC) ** 6)
_B = math.sqrt(0.5)

# cumulative tile boundaries as fractions of total width (small first tile
# for pipeline ramp, small last tile for a short tail)
TILE_FRACS = [0.045, 0.16, 0.37, 0.60, 0.82, 1.0]
GP_TT = 0.35  # fraction of the bp multiply given to GpSimd (early tiles)
GP_FOLD = 0.35  # fraction of the fold add given to GpSimd (early tiles)
GP_TILES = (1, 2, 3)  # tiles whose tt/fold get a GpSimd slice


def _merge_runs(Lc: np.ndarray, max_runs: int = 7, max_cost: int = 60000):
    """Round some chunks' L up to the next-larger run's L to cut the number
    of distinct L values. Lc is non-increasing (sorted desc)."""
    Lc = Lc.copy()
    while True:
        uniq = sorted(set(int(x) for x in Lc), reverse=True)
        if len(uniq) <= max_runs:
            break
        best = None
        for i in range(1, len(uniq)):
            src = uniq[i]
            dst = uniq[i - 1]
            m = int(np.sum(Lc == src))
            cost = m * CH * (dst - src)
            if best is None or cost < best[0]:
                best = (cost, src, dst)
        if best[0] > max_cost:
            break
        Lc[Lc == best[1]] = best[2]
    return Lc


def _chunk_geometry(idx: np.ndarray, n_atoms: int):
    counts = np.bincount(idx, minlength=n_atoms).astype(np.int64)
    perm = np.argsort(idx, kind="stable")
    starts = np.zeros(n_atoms + 1, np.int64)
    starts[1:] = np.cumsum(counts)
    q = ((counts + PAD_MULT - 1) // PAD_MULT) * PAD_MULT
    order = np.argsort(-q, kind="stable")
    n_chunks = (n_atoms + CH - 1) // CH
    n_pad = n_chunks * CH
    order_pad = np.full(n_pad, -1, np.int64)
    order_pad[:n_atoms] = order
    qs = np.where(order_pad >= 0, q[np.maximum(order_pad, 0)], 0)
    Lc = np.maximum(qs.reshape(n_chunks, CH).max(axis=1), PAD_MULT)
    Lc = _merge_runs(Lc)
    Lp = Lc + N_FIX  # even
    return counts, perm, starts, order_pad, Lc, Lp, n_chunks


def _tile_plan(Lp):
    """Group chunks into device tiles at TILE_FRACS boundaries.

    Returns list of tiles (col_start, F, runs);
    runs = [(w_off, Lh, m, out_col)] over the tile's FOLDED layout, where
    Lh = Lp/2 and w_off is the column offset inside the folded half-width.
    """
    n = len(Lp)
    total = sum(Lp)
    bounds = []
    c0 = 0
    col = 0
    fi = 0
    for i in range(n):
        col += Lp[i]
        if fi < len(TILE_FRACS) - 1 and col >= TILE_FRACS[fi] * total:
            bounds.append((c0, i + 1))
            c0 = i + 1
            fi += 1
    if c0 < n:
        bounds.append((c0, n))
    tiles = []
    col = 0
    for c0, c1 in bounds:
        runs = []
        off = 0  # offset in folded half-width units
        j = c0
        while j < c1:
            k = j
            while k < c1 and Lp[k] == Lp[j]:
                k += 1
            runs.append((off, Lp[j] // 2, k - j, j))
            off += (Lp[j] // 2) * (k - j)
            j = k
        tiles.append((col, 2 * off, runs))
        col += 2 * off
    return tiles


def _build_layout(idx: np.ndarray, n_atoms: int, dist: np.ndarray):
    """Pack pairs into per-core partition-major fp16 tiles (folded order).

    Returns (packed, atom_of, Lp, n_chunks, tiles).
    """
    counts, perm, starts, order_pad, Lc, Lp, n_chunks = _chunk_geometry(
        idx, n_atoms
    )
    tiles = _tile_plan([int(x) for x in Lp])
    F_total = sum(F for _, F, _ in tiles)

    # fixup distances: each of the two slots contributes bp = -Lc*e0/4
    vfix = _B + np.sqrt(0.5 - Lc * _E0 / 4.0)
    dfix = (math.sqrt(2.0) / vfix) ** (1.0 / 6.0)

    dist_sorted = dist[perm].astype(np.float16)
    packed = np.full((N_CORES, P, F_total), np.float16(RC), np.float16)
    Lmax = int(Lc.max())
    offs_max = np.arange(Lmax)
    for tcol, Ft, runs in tiles:
        half = Ft // 2
        for w_off, Lh, m, j0 in runs:
            for j in range(j0, j0 + m):
                a = order_pad[j * CH : (j + 1) * CH]
                L = int(Lc[j])
                cnt = np.where(a >= 0, counts[np.maximum(a, 0)], 0)
                offs = offs_max[:L][None, :]
                valid = offs < cnt[:, None]
                src = starts[np.maximum(a, 0)][:, None] + offs
                block = np.full((CH, L + N_FIX), np.float16(RC), np.float16)
                block[:, :L][valid] = dist_sorted[src[valid]]
                block[:, L:] = np.float16(dfix[j])
                blk = block.reshape(N_CORES, P, L + N_FIX)
                o = tcol + w_off + (j - j0) * Lh
                packed[:, :, o : o + Lh] = blk[:, :, :Lh]
                packed[:, :, half + o : half + o + Lh] = blk[:, :, Lh:]
    atom_of = order_pad.reshape(n_chunks, N_CORES, P)
    return packed, atom_of, [int(x) for x in Lp], n_chunks, tiles


def _build_bass_program(tiles, F_total, n_chunks):
    import concourse.bass as bass
    import concourse.tile as tile
    from concourse import bacc, mybir

    f32 = mybir.dt.float32
    f16 = mybir.dt.float16
    AF = mybir.ActivationFunctionType
    OP = mybir.AluOpType

    nc = bacc.Bacc(
        "TRN2",
        target_bir_lowering=False,
        debug=False,
        enable_asserts=False,
        num_devices=N_CORES,
    )
    din = nc.dram_tensor("dist_packed", [P, F_total], f16, kind="ExternalInput")
    dout = nc.dram_tensor("en_out", [P, n_chunks], f32, kind="ExternalOutput")

    # activation table set holding ln+exp together (one load for the whole
    # program instead of a 1.3us reload per function switch)
    set_id = 6
    try:
        from concourse.hw_specs import get_activation_tables

        for i, (_, funcs) in enumerate(get_activation_tables("TRN2").items()):
            if AF.Ln in funcs and AF.Exp in funcs:
                set_id = i
                break
    except Exception:
        pass

    ln_sqrt2 = 0.5 * math.log(2.0)
    n_tiles = len(tiles)

    with tile.TileContext(nc) as tc:
        with (
            tc.tile_pool(name="io", bufs=3) as io_pool,
            tc.tile_pool(name="t", bufs=3) as tpool,
            tc.tile_pool(name="v", bufs=6) as vpool,
            tc.tile_pool(name="u", bufs=4) as upool,
            tc.tile_pool(name="w", bufs=3) as wpool,
            tc.tile_pool(name="acc", bufs=1) as acc_pool,
        ):
            atl = mybir.InstLoadActFuncSet(
                name=nc.get_next_instruction_name(),
                ins=[],
                outs=[],
                act_func_set_id=set_id,
            )
            nc.scalar.add_instruction(atl)
            out_raw = acc_pool.tile([P, n_chunks], f32, tag="out_raw")
            lbias = acc_pool.tile([P, 1], f32, tag="lbias")
            nc.vector.memset(lbias[:], ln_sqrt2)
            for ti, (col, F, runs) in enumerate(tiles):
                half = F // 2
                use_gp = ti in GP_TILES
                d = io_pool.tile([P, F], f16, tag="d")
                nc.sync.dma_start(d[:], din.ap()[:, col : col + F])
                # t = ln(d) at f32 (exp amplifies ln error 6x)
                t = tpool.tile([P, F], f32, tag="t")
                nc.scalar.activation(t[:], d[:], AF.Ln)
                # v = sqrt2*d^-6 in fp16 in its own deep pool so the Scalar
                # engine never waits on Vector/GpSimd consumers (WAR)
                v = vpool.tile([P, F], f16, tag="v")
                nc.scalar.activation(
                    v[:], t[:], AF.Exp, bias=lbias[:], scale=-6.0
                )
                # bp = (v - 2b)*v ; en/2 = bp - e0/2 (constant folded into
                # the per-chunk fixup slots).  ts runs 4x, tt 2x; a slice
                # of tt goes to the otherwise-idle GpSimd engine.
                u = upool.tile([P, F], f16, tag="u")
                nc.vector.tensor_scalar(u[:], v[:], 2.0 * _B, None, OP.subtract)
                sp = (int(F * (1.0 - GP_TT)) & ~1) if use_gp else F
                nc.vector.tensor_tensor(
                    v[:, :sp], u[:, :sp], v[:, :sp], OP.mult
                )
                if sp < F:
                    nc.gpsimd.tensor_tensor(
                        v[:, sp:], u[:, sp:], v[:, sp:], OP.mult
                    )
                # fold 2:1 with a single full-width add (layout is mirrored)
                w = wpool.tile([P, half], f16, tag="w")
                hsp = (int(half * (1.0 - GP_FOLD)) & ~1) if use_gp else half
                nc.vector.tensor_tensor(
                    w[:, :hsp], v[:, :hsp], v[:, half : half + hsp], OP.add
                )
                if hsp < half:
                    nc.gpsimd.tensor_tensor(
                        w[:, hsp:], v[:, hsp:half], v[:, half + hsp :], OP.add
                    )
                c0 = runs[0][3]
                c1 = runs[-1][3] + runs[-1][2]
                for w_off, Lh, m, out_col in runs:
                    nc.vector.tensor_reduce(
                        out_raw[:, out_col : out_col + m],
                        w[:, w_off : w_off + m * Lh].rearrange(
                            "p (b l) -> p b l", l=Lh
                        ),
                        axis=mybir.AxisListType.X,
                        op=OP.add,
                    )
                nc.sync.dma_start(dout.ap()[:, c0:c1], out_raw[:, c0:c1])
    nc.compile()
    return nc


def _prepare(inputs):
    dist = np.ascontiguousarray(np.asarray(inputs["dist"], dtype=np.float32))
    ind_2 = np.asarray(inputs["ind_2"])
    n_atoms = int(np.asarray(inputs["ind_1"]).shape[0])
    idx = ind_2[:, 0].astype(np.int64)

    packed, atom_of, Lp, n_chunks, tiles = _build_layout(idx, n_atoms, dist)
    F_total = packed.shape[2]
    in_maps = [
        {"dist_packed": np.ascontiguousarray(packed[c])} for c in range(N_CORES)
    ]
    nc = _build_bass_program(tiles, F_total, n_chunks)
    return nc, in_maps, (atom_of, n_atoms)


def _finish(res, meta):
    atom_of, n_atoms = meta
    out_full = np.zeros(n_atoms, np.float32)
    for c in range(N_CORES):
        dev = res.results[c]["en_out"]  # [P, n_chunks]
        a = atom_of[:, c, :]  # [n_chunks, P]
        valid = a >= 0
        out_full[a[valid]] = dev.T[valid]
    return out_full


def kernel(**inputs) -> np.ndarray:
    nc, in_maps, meta = _prepare(inputs)

    from concourse import bass_utils

    res = bass_utils.run_bass_kernel_spmd(
        nc, in_maps, core_ids=list(range(N_CORES))
    )
    return _finish(res, meta)
